# revision 41
# baseline (speedup 1.0000x reference)
"""Bass/Tile Trainium2 kernel for nn_Attention_37538014167301.

GQA attention layer (B=2, S=2048, E=2048, H=16, KVH=4, D=128) with RoPE and
causal softmax, sharded over 8 NeuronCores: batch (2-way) x head-group
(4-way tensor parallel).  Core c handles batch b=c//4 and q heads
{g, g+4, g+8, g+12} with g=c%4; under torch-style .repeat (jnp.tile) GQA,
those q heads all use kv head g, so each core needs exactly one kv head.

Everything on device is kept in transposed [dim, seq] layout so that every
matmul contracts over the partition axis:
  - projections compute Q^T/K^T/V^T = W @ x^T directly,
  - RoPE is applied in [d, s] layout using a pair-swap permutation matmul
    plus elementwise ops against host-precomputed cos/sin tables,
  - scores are computed transposed (P^T[sk, sq]) so that P^T can feed the
    attention*V matmul with V in natural [sk, d] layout (V is transposed
    on-chip via the PE transpose path),
  - the softmax denominator is a ones-vector matmul accumulated alongside
    attention*V, and the normalization happens on the output tile,
  - the output projection accumulates y^T[f, s] per core; the host sums the
    four head-group partials per batch.

Matmuls run as float32r (full PE rate for free dim >= 256); fp32 data is
bitcast, accumulation stays fp32 in PSUM.
"""

import math

import numpy as np

B, S, E = 2, 2048, 2048
H, KVH = 16, 4
D = E // H  # 128
N_CORES = 8
GROUPS = 4  # head groups (tensor-parallel degree per batch)
HQ = H // GROUPS  # q heads per core


# ---------------------------------------------------------------------------
# Device program
# ---------------------------------------------------------------------------

def emit_attention(tc, io, S_=S, E_=E, HQ_=HQ, D_=D, CH=512, XSPLIT=4):
    """Emit the per-core attention program into TileContext tc.

    io: dict of DRAM APs: xT, wqT, wkT, wvT, woT, cosd, sind, swp, yT
    """
    import concourse.mybir as mybir
    from concourse.masks import make_identity

    nc = tc.nc
    f32 = mybir.dt.float32
    f32r = mybir.dt.float32r

    NE = E_ // 128        # contraction tiles over e
    XSPLIT = min(XSPLIT, NE)
    NSQ = S_ // CH        # q chunks
    R = CH // 128         # sk tiles per q chunk width
    NSK = S_ // 128       # sk tiles
    DQ = HQ_ * D_
    scale = 1.0 / math.sqrt(D_)

    xT, wqT, wkT, wvT, woT = io["xT"], io["wqT"], io["wkT"], io["wvT"], io["woT"]
    cosd, sind, swp, yT = io["cosd"], io["sind"], io["swp"], io["yT"]

    from contextlib import ExitStack

    with ExitStack() as ctx:
        const = ctx.enter_context(tc.tile_pool(name="const", bufs=1))
        persist = ctx.enter_context(tc.tile_pool(name="persist", bufs=1))
        pacc = ctx.enter_context(tc.tile_pool(name="pacc", bufs=2, space="PSUM"))
        pden = ctx.enter_context(tc.tile_pool(name="pden", bufs=2, space="PSUM"))

        cos_sb = const.tile([D_, S_], f32, name="cos_sb")
        nc.sync.dma_start(out=cos_sb, in_=cosd)
        sin_sb = const.tile([D_, S_], f32, name="sin_sb")
        nc.sync.dma_start(out=sin_sb, in_=sind)
        swp_sb = const.tile([D_, D_], f32r, name="swp_sb")
        nc.sync.dma_start(out=swp_sb, in_=swp)
        ones_f = const.tile([128, 1], f32, name="ones_f")
        nc.gpsimd.memset(ones_f, 1.0)
        ones_sb = const.tile([128, 1], f32r, name="ones_sb")
        nc.vector.tensor_copy(out=ones_sb, in_=ones_f)
        ident_sb = const.tile([128, 128], f32, name="ident_sb")
        make_identity(nc, ident_sb)

        q_rot = [persist.tile([D_, S_], f32r, name=f"qrot{m}") for m in range(HQ_)]
        k_rot = persist.tile([D_, S_], f32r, name="k_rot")
        v_nat = persist.tile([128, NSK, D_], f32r, name="v_nat")

        # ------------------------------------------------------------------
        # Phase A: projections + rope, chunk by chunk over s
        # ------------------------------------------------------------------
        with ExitStack() as actx:
            wpool = actx.enter_context(tc.tile_pool(name="wpool", bufs=1))
            xpool = actx.enter_context(tc.tile_pool(name="xpool", bufs=2 * XSPLIT))
            work = actx.enter_context(tc.tile_pool(name="awork", bufs=3))
            pmm = actx.enter_context(tc.tile_pool(name="pmm", bufs=4, space="PSUM"))

            # weights arrive as [E, n] = [(t p), n] -> [p, t, n] sbuf layout.
            # Small K/V weights first so the first projection group's inputs
            # (wk + x chunk 0) aren't stuck behind the 4MB wq transfer.
            wk_sb = wpool.tile([128, NE, D_], f32r, name="wk_sb")
            nc.sync.dma_start(out=wk_sb,
                              in_=wkT.rearrange("(t p) n -> p t n", p=128))
            wv_sb = wpool.tile([128, NE, D_], f32r, name="wv_sb")
            nc.sync.dma_start(out=wv_sb,
                              in_=wvT.rearrange("(t p) n -> p t n", p=128))

            xT_r = xT.rearrange("(t p) s -> p t s", p=128)  # [128, NE, S]
            TG = NE // XSPLIT  # e-tiles per x DMA
            wq_sb = wpool.tile([128, NE, DQ], f32r, name="wq_sb")
            wqT_r = wqT.rearrange("(t p) n -> p t n", p=128)

            for j in range(NSQ):
                ssl = slice(CH * j, CH * (j + 1))
                x_chunks = []
                for u in range(XSPLIT):
                    xc = xpool.tile([128, TG, CH], f32r, name="xc", tag="xc")
                    nc.sync.dma_start(
                        out=xc, in_=xT_r[:, TG * u:TG * (u + 1), ssl])
                    x_chunks.append(xc)
                x_tiles = [x_chunks[t // TG][:, t % TG, :] for t in range(NE)]
                if j == 0:
                    # wq arrives after the first x chunk, in 4 e-tile groups
                    for u in range(XSPLIT):
                        nc.sync.dma_start(
                            out=wq_sb[:, TG * u:TG * (u + 1), :],
                            in_=wqT_r[:, TG * u:TG * (u + 1), :])

                def project(w_slices, n):
                    ps = pmm.tile([128, CH], f32, name="ps_proj", tag="mm")
                    for t in range(NE):
                        nc.tensor.matmul(
                            ps[:n, :], w_slices(t), x_tiles[t],
                            start=(t == 0), stop=(t == NE - 1),
                        )
                    return ps

                def rope(ps, dst):
                    # dst[:, ssl] = ps * cos + (SWAP @ ps) * sin_signed
                    p_sb = work.tile([D_, CH], f32r, name="rope_in", tag="rope_in")
                    nc.vector.tensor_copy(out=p_sb, in_=ps[:D_, :])
                    ps2 = pmm.tile([128, CH], f32, name="ps_swap", tag="mm")
                    nc.tensor.matmul(ps2[:D_, :], swp_sb, p_sb,
                                     start=True, stop=True)
                    t1 = work.tile([D_, CH], f32, name="rope_t1", tag="rope_t1")
                    nc.vector.tensor_mul(t1, p_sb.bitcast(f32), cos_sb[:, ssl])
                    t2 = work.tile([D_, CH], f32, name="rope_t2", tag="rope_t2")
                    nc.vector.tensor_mul(t2, ps2[:D_, :], sin_sb[:, ssl])
                    nc.vector.tensor_add(dst[:, ssl], t1, t2)

                # K
                ps = project(lambda t: wk_sb[:, t, :], D_)
                rope(ps, k_rot)
                # V: copy to sbuf, then PE-transpose each 128x128 block
                ps = project(lambda t: wv_sb[:, t, :], D_)
                vt_sb = work.tile([D_, CH], f32, name="vt_sb", tag="vt_sb")
                nc.vector.tensor_copy(out=vt_sb, in_=ps[:D_, :])
                for u in range(R):
                    pvt = pmm.tile([128, CH], f32, name="ps_vt", tag="mm")
                    nc.tensor.transpose(pvt[:, :D_], vt_sb[:, 128 * u:128 * (u + 1)],
                                        ident_sb)
                    nc.vector.tensor_copy(out=v_nat[:, R * j + u, :], in_=pvt[:, :D_])
                # Q heads
                for m in range(HQ_):
                    ps = project(lambda t: wq_sb[:, t, D_ * m:D_ * (m + 1)], D_)
                    rope(ps, q_rot[m])

        # ------------------------------------------------------------------
        # Phase B: attention per (head, q chunk); Phase C: output projection
        # ------------------------------------------------------------------
        with ExitStack() as bctx:
            bpool = bctx.enter_context(tc.tile_pool(name="bpool", bufs=1))
            pwork = bctx.enter_context(tc.tile_pool(name="pwork", bufs=4))
            nwork = bctx.enter_context(tc.tile_pool(name="nwork", bufs=2))
            psc_pool = bctx.enter_context(
                tc.tile_pool(name="psc", bufs=2, space="PSUM"))

            o_sb = [bpool.tile([D_, S_], f32r, name=f"osb{m}") for m in range(HQ_)]

            wopool = bctx.enter_context(tc.tile_pool(name="wopool", bufs=1))
            ywork = bctx.enter_context(tc.tile_pool(name="ywork", bufs=3))
            wo_sb = wopool.tile([128, HQ_, E_], f32r, name="wo_sb")
            for m in range(HQ_):
                nc.sync.dma_start(out=wo_sb[:, m, :], in_=woT[128 * m:128 * (m + 1), :])

            assert HQ_ % 2 == 0
            for j in range(NSQ):
                ssl = slice(CH * j, CH * (j + 1))
                for hp in range(HQ_ // 2):
                    m0, m1 = 2 * hp, 2 * hp + 1
                    n_i = R * (j + 1)
                    po0 = pacc.tile([128, CH], f32, name="po0", tag="acc")
                    po1 = pacc.tile([128, CH], f32, name="po1", tag="acc")
                    pd0 = pden.tile([1, CH], f32, name="pd0", tag="den")
                    pd1 = pden.tile([1, CH], f32, name="pd1", tag="den")
                    for i in range(n_i):
                        # diagonal tiles: columns < 128*u are fully below the
                        # causal boundary -> skip them in scores/exp/PV/den;
                        # only the 128-wide strip [128u, 128(u+1)) needs a mask
                        diag = i >= R * j
                        u = i - R * j if diag else 0
                        lo = 128 * u
                        csl = slice(lo, CH)
                        qsl = slice(CH * j + lo, CH * (j + 1))
                        # transposed scores for both heads into one 2-bank psum
                        psc = psc_pool.tile([128, 2, CH], f32, name="psc",
                                            tag="sc")
                        nc.tensor.matmul(
                            psc[:, 0, csl], k_rot[:, 128 * i:128 * (i + 1)],
                            q_rot[m0][:, qsl], start=True, stop=True)
                        nc.tensor.matmul(
                            psc[:, 1, csl], k_rot[:, 128 * i:128 * (i + 1)],
                            q_rot[m1][:, qsl], start=True, stop=True)
                        p_sb = pwork.tile([128, 2, CH], f32r, name="p_sb",
                                          tag="p_sb")
                        nc.scalar.activation(out=p_sb[:, :, csl],
                                             in_=psc[:, :, csl],
                                             func=mybir.ActivationFunctionType.Exp,
                                             scale=scale)
                        if diag:
                            # keep where sk <= sq, i.e. strip col c' >= p
                            nc.gpsimd.affine_select(
                                out=p_sb[:, :, lo:lo + 128],
                                in_=p_sb[:, :, lo:lo + 128],
                                compare_op=mybir.AluOpType.is_ge,
                                fill=0.0,
                                base=0,
                                pattern=[[0, 2], [1, 128]],
                                channel_multiplier=-1,
                            )
                        last = i == n_i - 1
                        nc.tensor.matmul(po0[:, csl], v_nat[:, i, :],
                                         p_sb[:, 0, csl],
                                         start=(i == 0), stop=last)
                        nc.tensor.matmul(po1[:, csl], v_nat[:, i, :],
                                         p_sb[:, 1, csl],
                                         start=(i == 0), stop=last)
                        nc.tensor.matmul(pd0[:, csl], ones_sb,
                                         p_sb[:, 0, csl],
                                         start=(i == 0), stop=last)
                        nc.tensor.matmul(pd1[:, csl], ones_sb,
                                         p_sb[:, 1, csl],
                                         start=(i == 0), stop=last)
                    for mm, po, pd in ((m0, po0, pd0), (m1, po1, pd1)):
                        # copy unnormalized out to sbuf on ACT so the psum
                        # accumulator frees without waiting for the
                        # recip/broadcast chain
                        ou = nwork.tile([D_, CH], f32, name="ou", tag="ou")
                        nc.scalar.activation(
                            out=ou, in_=po[:D_, :],
                            func=mybir.ActivationFunctionType.Copy)
                        recip = nwork.tile([1, CH], f32, name="recip",
                                           tag="recip")
                        nc.vector.reciprocal(out=recip, in_=pd)
                        rbc = nwork.tile([128, CH], f32, name="rbc", tag="rbc")
                        nc.gpsimd.partition_broadcast(rbc, recip)
                        nc.vector.tensor_mul(o_sb[mm][:, ssl], ou,
                                             rbc[:D_, :])

                # output projection for this s-chunk (pipelines behind
                # attention of the next chunk)
                for tf in range(NE):
                    fsl = slice(128 * tf, 128 * (tf + 1))
                    py = pacc.tile([128, CH], f32, name="py", tag="acc")
                    for m in range(HQ_):
                        nc.tensor.matmul(py, wo_sb[:, m, fsl],
                                         o_sb[m][:, ssl],
                                         start=(m == 0), stop=(m == HQ_ - 1))
                    y_sb = ywork.tile([128, CH], f32, name="y_sb", tag="y_sb")
                    nc.any.tensor_copy(out=y_sb, in_=py)
                    nc.sync.dma_start(out=yT[fsl, ssl], in_=y_sb)


def emit_attention_v2(tc, io, S_=S, E_=E, HQ_=HQ, D_=D, CH=512, XSPLIT=8,
                      phases="ABC"):
    """Fused emission: per s-chunk, projections + rope + attention are
    interleaved so PE has dense work across what were phase boundaries.
    Output projection stays a final phase (SBUF can't hold wo alongside the
    projection working set).

    PSUM budget (8 banks): psc pool 2 bufs x [128,2,CH] (4) + pacc 2 + pden 2.
    Projection accumulators, rope-swap outputs and V-transposes share psc
    slots in head pairs.
    """
    import concourse.mybir as mybir
    from concourse.masks import make_identity

    nc = tc.nc
    f32 = mybir.dt.float32
    f32r = mybir.dt.float32r

    NE = E_ // 128
    XSPLIT = min(XSPLIT, NE)
    NSQ = S_ // CH
    R = CH // 128
    NSK = S_ // 128
    DQ = HQ_ * D_
    NP = HQ_ // 2  # head pairs
    scale = 1.0 / math.sqrt(D_)
    assert HQ_ % 2 == 0

    xT, wqT, wkT, wvT, woT = io["xT"], io["wqT"], io["wkT"], io["wvT"], io["woT"]
    cosd, sind, swp, yT = io["cosd"], io["sind"], io["swp"], io["yT"]

    from contextlib import ExitStack

    with ExitStack() as ctx:
        const = ctx.enter_context(tc.tile_pool(name="const", bufs=1))
        persist = ctx.enter_context(tc.tile_pool(name="persist", bufs=1))
        qpool = ctx.enter_context(tc.tile_pool(name="qpool", bufs=2))
        cpool = ctx.enter_context(tc.tile_pool(name="cpool", bufs=2))
        pacc = ctx.enter_context(tc.tile_pool(name="pacc", bufs=2, space="PSUM"))
        pden = ctx.enter_context(tc.tile_pool(name="pden", bufs=2, space="PSUM"))
        psc_pool = ctx.enter_context(
            tc.tile_pool(name="psc", bufs=2, space="PSUM"))

        swp_sb = const.tile([D_, D_], f32r, name="swp_sb")
        nc.sync.dma_start(out=swp_sb, in_=swp)
        ones_f = const.tile([128, 1], f32, name="ones_f")
        nc.gpsimd.memset(ones_f, 1.0)
        ones_sb = const.tile([128, 1], f32r, name="ones_sb")
        nc.vector.tensor_copy(out=ones_sb, in_=ones_f)
        ident_sb = const.tile([128, 128], f32, name="ident_sb")
        make_identity(nc, ident_sb)

        k_rot = persist.tile([D_, S_], f32r, name="k_rot")
        v_nat = persist.tile([128, NSK, D_], f32r, name="v_nat")
        o_sb = [persist.tile([D_, S_], f32r, name=f"osb{m}")
                for m in range(HQ_)]

        with ExitStack() as actx:
            wpool = actx.enter_context(tc.tile_pool(name="wpool", bufs=1))
            xpool = actx.enter_context(tc.tile_pool(name="xpool", bufs=XSPLIT))
            work = actx.enter_context(tc.tile_pool(name="awork", bufs=2))
            pwork = actx.enter_context(tc.tile_pool(name="pwork", bufs=3))
            nwork = actx.enter_context(tc.tile_pool(name="nwork", bufs=2))

            # K weights first (in 4 sub-DMAs so the first projection matmul
            # starts as soon as wk part 0 + x part 0 land); wv/wq follow the
            # first x chunk since their projections run later
            TG = NE // XSPLIT
            wk_sb = wpool.tile([128, NE, D_], f32r, name="wk_sb")
            wkT_r = wkT.rearrange("(t p) n -> p t n", p=128)
            for u in range(XSPLIT):
                nc.sync.dma_start(out=wk_sb[:, TG * u:TG * (u + 1), :],
                                  in_=wkT_r[:, TG * u:TG * (u + 1), :])
            wv_sb = wpool.tile([128, NE, D_], f32r, name="wv_sb")
            wvT_r = wvT.rearrange("(t p) n -> p t n", p=128)

            xT_r = xT.rearrange("(t p) s -> p t s", p=128)
            wq_sb = wpool.tile([128, NE, DQ], f32r, name="wq_sb")
            wqT_r = wqT.rearrange("(t p) n -> p t n", p=128)

            for j in range(NSQ):
                ssl = slice(CH * j, CH * (j + 1))
                x_chunks = []
                for u in range(XSPLIT):
                    xc = xpool.tile([128, TG, CH], f32r, name="xc", tag="xc")
                    nc.sync.dma_start(
                        out=xc, in_=xT_r[:, TG * u:TG * (u + 1), ssl])
                    x_chunks.append(xc)
                x_tiles = [x_chunks[t // TG][:, t % TG, :] for t in range(NE)]
                if j == 0:
                    nc.sync.dma_start(out=wv_sb, in_=wvT_r)
                    for u in range(XSPLIT):
                        nc.sync.dma_start(
                            out=wq_sb[:, TG * u:TG * (u + 1), :],
                            in_=wqT_r[:, TG * u:TG * (u + 1), :])

                cos_c = cpool.tile([D_, CH], f32, name="cos_c", tag="cos_c")
                nc.sync.dma_start(out=cos_c, in_=cosd[:, ssl])
                sin_c = cpool.tile([D_, CH], f32, name="sin_c", tag="sin_c")
                nc.sync.dma_start(out=sin_c, in_=sind[:, ssl])

                # --- K+V projections into one paired psum slot ---
                pkv = psc_pool.tile([128, 2, CH], f32, name="pkv", tag="sc")
                for t in range(NE):
                    nc.tensor.matmul(pkv[:, 0, :], wk_sb[:, t, :], x_tiles[t],
                                     start=(t == 0), stop=(t == NE - 1))
                for t in range(NE):
                    nc.tensor.matmul(pkv[:, 1, :], wv_sb[:, t, :], x_tiles[t],
                                     start=(t == 0), stop=(t == NE - 1))

                # --- K rope + V transpose reuse the pkv slot banks ---
                rink = work.tile([D_, 2, CH], f32r, name="rin", tag="rin")
                nc.vector.tensor_copy(out=rink[:, 0, :], in_=pkv[:, 0, :])
                nc.tensor.matmul(pkv[:, 0, :], swp_sb, rink[:, 0, :],
                                 start=True, stop=True)
                t1k = work.tile([D_, 2, CH], f32, name="t1", tag="t1")
                nc.vector.tensor_mul(t1k[:, 0, :], rink[:, 0, :].bitcast(f32),
                                     cos_c)
                t2k = work.tile([D_, 2, CH], f32, name="t2", tag="t2")
                nc.vector.tensor_mul(t2k[:, 0, :], pkv[:, 0, :], sin_c)
                nc.vector.tensor_add(k_rot[:, ssl], t1k[:, 0, :], t2k[:, 0, :])

                vt_sb = work.tile([D_, CH], f32, name="vt_sb", tag="vt_sb")
                nc.vector.tensor_copy(out=vt_sb, in_=pkv[:, 1, :])
                for u in range(R):
                    nc.tensor.transpose(pkv[:, 1, 128 * u:128 * (u + 1)],
                                        vt_sb[:, 128 * u:128 * (u + 1)],
                                        ident_sb)
                    nc.vector.tensor_copy(out=v_nat[:, R * j + u, :],
                                          in_=pkv[:, 1, 128 * u:128 * (u + 1)])

                # --- Q projections + rope, in head pairs ---
                qp = []
                for p in range(NP):
                    m0 = 2 * p
                    pq = psc_pool.tile([128, 2, CH], f32, name="pq", tag="sc")
                    for h in range(2):
                        wsl = slice(D_ * (m0 + h), D_ * (m0 + h + 1))
                        for t in range(NE):
                            nc.tensor.matmul(
                                pq[:, h, :], wq_sb[:, t, wsl], x_tiles[t],
                                start=(t == 0), stop=(t == NE - 1))
                    rin = work.tile([D_, 2, CH], f32r, name="rin", tag="rin")
                    nc.vector.tensor_copy(out=rin, in_=pq[:D_, :, :])
                    for h in range(2):
                        nc.tensor.matmul(pq[:D_, h, :], swp_sb, rin[:, h, :],
                                         start=True, stop=True)
                    cos_b = cos_c[:, None, :].broadcast_to([D_, 2, CH])
                    sin_b = sin_c[:, None, :].broadcast_to([D_, 2, CH])
                    t1 = work.tile([D_, 2, CH], f32, name="t1", tag="t1")
                    nc.vector.tensor_mul(t1, rin.bitcast(f32), cos_b)
                    t2 = work.tile([D_, 2, CH], f32, name="t2", tag="t2")
                    nc.vector.tensor_mul(t2, pq[:D_, :, :], sin_b)
                    qrot = qpool.tile([D_, 2, CH], f32r, name=f"qrot{p}",
                                      tag=f"qrot{p}")
                    nc.vector.tensor_add(qrot, t1, t2)
                    qp.append(qrot)

                # --- attention for this chunk ---
                for p in (range(NP) if "B" in phases else ()):
                    m0, m1 = 2 * p, 2 * p + 1
                    n_i = R * (j + 1)
                    po0 = pacc.tile([128, CH], f32, name="po0", tag="acc")
                    po1 = pacc.tile([128, CH], f32, name="po1", tag="acc")
                    pd0 = pden.tile([1, CH], f32, name="pd0", tag="den")
                    pd1 = pden.tile([1, CH], f32, name="pd1", tag="den")
                    for i in range(n_i):
                        diag = i >= R * j
                        u = i - R * j if diag else 0
                        lo = 128 * u
                        csl = slice(lo, CH)
                        W = CH - lo
                        # merge the head pair into one matmul when the
                        # combined moving size fits the 512 fp32 limit
                        merged = False  # CoreSim can't validate strided pair matmuls
                        psc = psc_pool.tile([128, 2, CH], f32, name="psc",
                                            tag="sc")
                        ksl = k_rot[:, 128 * i:128 * (i + 1)]
                        if merged:
                            nc.tensor.matmul(psc[:, :, csl], ksl,
                                             qp[p][:, :, csl],
                                             start=True, stop=True)
                        else:
                            nc.tensor.matmul(psc[:, 0, csl], ksl,
                                             qp[p][:, 0, csl],
                                             start=True, stop=True)
                            nc.tensor.matmul(psc[:, 1, csl], ksl,
                                             qp[p][:, 1, csl],
                                             start=True, stop=True)
                        p_sb = pwork.tile([128, 2, CH], f32r, name="p_sb",
                                          tag="p_sb")
                        nc.scalar.activation(
                            out=p_sb[:, :, csl], in_=psc[:, :, csl],
                            func=mybir.ActivationFunctionType.Exp, scale=scale)
                        if diag:
                            nc.gpsimd.affine_select(
                                out=p_sb[:, :, lo:lo + 128],
                                in_=p_sb[:, :, lo:lo + 128],
                                compare_op=mybir.AluOpType.is_ge,
                                fill=0.0, base=0,
                                pattern=[[0, 2], [1, 128]],
                                channel_multiplier=-1,
                            )
                        last = i == n_i - 1
                        if merged:
                            nc.tensor.matmul(po0[:, csl], v_nat[:, i, :],
                                             p_sb[:, 0, csl],
                                             start=(i == 0), stop=last)
                            nc.tensor.matmul(pd0[:, csl], ones_sb,
                                             p_sb[:, 0, csl],
                                             start=(i == 0), stop=last)
                            nc.tensor.matmul(pd1[:, csl], ones_sb,
                                             p_sb[:, 1, csl],
                                             start=(i == 0), stop=last)
                        else:
                            # stop=True closes each matmul's psum group so the
                            # paired po tile never has two pending groups;
                            # has_written persists, so accumulation continues
                            nc.tensor.matmul(po0[:, csl], v_nat[:, i, :],
                                             p_sb[:, 0, csl],
                                             start=(i == 0), stop=last)
                            nc.tensor.matmul(po1[:, csl], v_nat[:, i, :],
                                             p_sb[:, 1, csl],
                                             start=(i == 0), stop=last)
                            nc.tensor.matmul(pd0[:, csl], ones_sb,
                                             p_sb[:, 0, csl],
                                             start=(i == 0), stop=last)
                            nc.tensor.matmul(pd1[:, csl], ones_sb,
                                             p_sb[:, 1, csl],
                                             start=(i == 0), stop=last)
                    for mm, po, pd in ((m0, po0, pd0), (m1, po1, pd1)):
                        ou = nwork.tile([D_, CH], f32, name="ou", tag="ou")
                        nc.scalar.activation(
                            out=ou, in_=po[:D_, :],
                            func=mybir.ActivationFunctionType.Copy)
                        recip = nwork.tile([1, CH], f32, name="recip",
                                           tag="recip")
                        nc.vector.reciprocal(out=recip, in_=pd)
                        rbc = nwork.tile([128, CH], f32, name="rbc", tag="rbc")
                        nc.gpsimd.partition_broadcast(rbc, recip)
                        nc.vector.tensor_mul(o_sb[mm][:, ssl], ou,
                                             rbc[:D_, :])

        # --- output projection (phase C) ---
        if "C" not in phases:
            return
        with ExitStack() as cctx:
            wopool = cctx.enter_context(tc.tile_pool(name="wopool", bufs=1))
            ywork = cctx.enter_context(tc.tile_pool(name="ywork", bufs=3))
            wo_sb = wopool.tile([128, HQ_, E_], f32r, name="wo_sb")
            for m in range(HQ_):
                nc.sync.dma_start(out=wo_sb[:, m, :],
                                  in_=woT[128 * m:128 * (m + 1), :])
            for tf in range(NE):
                fsl = slice(128 * tf, 128 * (tf + 1))
                for j in range(NSQ):
                    ssl = slice(CH * j, CH * (j + 1))
                    py = pacc.tile([128, CH], f32, name="py", tag="acc")
                    for m in range(HQ_):
                        nc.tensor.matmul(py, wo_sb[:, m, fsl],
                                         o_sb[m][:, ssl],
                                         start=(m == 0), stop=(m == HQ_ - 1))
                    y_sb = ywork.tile([128, CH], f32, name="y_sb", tag="y_sb")
                    nc.any.tensor_copy(out=y_sb, in_=py)
                    nc.sync.dma_start(out=yT[fsl, ssl], in_=y_sb)


def build_nc(S_=S, E_=E, HQ_=HQ, D_=D, CH=512, n_cores=N_CORES, reps=1,
             version=2, phases="ABC"):
    """Build and compile the per-core Bass program (same program on all cores).

    reps > 1 wraps the whole body in a hardware For_i loop (timing harness
    use only: amortizes host dispatch overhead across reps executions).
    """
    import concourse.mybir as mybir
    import concourse.tile as tile
    from concourse import bacc

    f32 = mybir.dt.float32
    f32r = mybir.dt.float32r
    DQ = HQ_ * D_

    nc = bacc.Bacc("TRN2", target_bir_lowering=False, debug=False,
                   num_devices=n_cores)
    io = {
        "xT": nc.dram_tensor("xT", [E_, S_], f32r, kind="ExternalInput").ap(),
        "wqT": nc.dram_tensor("wqT", [E_, DQ], f32r, kind="ExternalInput").ap(),
        "wkT": nc.dram_tensor("wkT", [E_, D_], f32r, kind="ExternalInput").ap(),
        "wvT": nc.dram_tensor("wvT", [E_, D_], f32r, kind="ExternalInput").ap(),
        "woT": nc.dram_tensor("woT", [DQ, E_], f32r, kind="ExternalInput").ap(),
        "cosd": nc.dram_tensor("cosd", [D_, S_], f32, kind="ExternalInput").ap(),
        "sind": nc.dram_tensor("sind", [D_, S_], f32, kind="ExternalInput").ap(),
        "swp": nc.dram_tensor("swp", [D_, D_], f32r, kind="ExternalInput").ap(),
        "yT": nc.dram_tensor("yT", [E_, S_], f32, kind="ExternalOutput").ap(),
    }
    def emit(tc):
        if version == 2:
            emit_attention_v2(tc, io, S_=S_, E_=E_, HQ_=HQ_, D_=D_, CH=CH,
                              phases=phases)
        else:
            emit_attention(tc, io, S_=S_, E_=E_, HQ_=HQ_, D_=D_, CH=CH)

    with tile.TileContext(nc) as tc:
        if reps == 1:
            emit(tc)
        else:
            with tc.For_i(0, reps, 1,
                          hint_engines=(mybir.EngineType.PE,
                                        mybir.EngineType.DVE,
                                        mybir.EngineType.Activation)):
                emit(tc)
    nc.compile()
    return nc


# ---------------------------------------------------------------------------
# Host-side sharding / gather
# ---------------------------------------------------------------------------

def round_fp32r(a):
    """Round fp32 array to fp32r (11-bit mantissa, RNE), keeping fp32 layout."""
    b = np.ascontiguousarray(a, dtype=np.float32).view(np.uint32)
    b = b + np.uint32(0x7FF) + ((b >> np.uint32(12)) & np.uint32(1))
    b &= np.uint32(0xFFFFF000)
    return b.view(np.float32)


def shard_inputs(x, wq_w, wk_w, wv_w, wo_w, freqs_cos, freqs_sin,
                 S_=S, E_=E, HQ_=HQ, D_=D, groups=GROUPS, n_batch=B):
    """Build per-core input maps (list ordered core 0..n-1, c = b*groups + g)."""
    x = np.asarray(x, dtype=np.float32)
    wq_w = np.asarray(wq_w, dtype=np.float32)
    wk_w = np.asarray(wk_w, dtype=np.float32)
    wv_w = np.asarray(wv_w, dtype=np.float32)
    wo_w = np.asarray(wo_w, dtype=np.float32)
    fc = np.asarray(freqs_cos, dtype=np.float32)  # [S, D/2]
    fs = np.asarray(freqs_sin, dtype=np.float32)

    cosd = np.repeat(fc.T, 2, axis=0)  # [D, S]
    sind = np.repeat(fs.T, 2, axis=0)
    sign = np.where(np.arange(D_) % 2 == 0, -1.0, 1.0).astype(np.float32)
    sind = sind * sign[:, None]
    swp = np.zeros((D_, D_), dtype=np.float32)
    idx = np.arange(0, D_, 2)
    swp[idx, idx + 1] = 1.0
    swp[idx + 1, idx] = 1.0

    in_maps = []
    for c in range(n_batch * groups):
        b, g = divmod(c, groups)
        heads = [g + groups * mm for mm in range(HQ_)]
        wq_rows = np.concatenate([wq_w[h * D_:(h + 1) * D_] for h in heads])  # [DQ, E]
        wo_cols = np.concatenate([wo_w[:, h * D_:(h + 1) * D_] for h in heads],
                                 axis=1)  # [E, DQ]
        in_maps.append({
            "xT": round_fp32r(x[b].T),
            "wqT": round_fp32r(wq_rows.T),
            "wkT": round_fp32r(wk_w[g * D_:(g + 1) * D_].T),
            "wvT": round_fp32r(wv_w[g * D_:(g + 1) * D_].T),
            "woT": round_fp32r(wo_cols.T),
            "cosd": cosd,
            "sind": sind,
            "swp": swp,  # exact 0/1 values, already valid fp32r
        })
    return in_maps


def gather_output(results, S_=S, E_=E, groups=GROUPS, n_batch=B):
    """results: list of per-core dicts with 'yT' [E, S] -> y [B, S, E]."""
    out = np.zeros((n_batch, S_, E_), dtype=np.float32)
    for c, res in enumerate(results):
        b = c // groups
        out[b] += res["yT"].T
    return out


# ---------------------------------------------------------------------------
# Per-core numpy reference (for validation in tests)
# ---------------------------------------------------------------------------

def percore_ref(xT, wqT, wkT, wvT, woT, fc, fs, HQ_=HQ, D_=D):
    x = xT.T.astype(np.float64)  # [S, E]
    S_ = x.shape[0]
    q = x @ wqT.astype(np.float64)  # [S, DQ]
    k = x @ wkT.astype(np.float64)  # [S, D]
    v = x @ wvT.astype(np.float64)

    def rope(t):  # [S, n, D]
        tr = t.reshape(*t.shape[:-1], -1, 2)
        xr, xi = tr[..., 0], tr[..., 1]
        c = fc[:, None, :]
        s = fs[:, None, :]
        orr = xr * c - xi * s
        oi = xr * s + xi * c
        return np.stack([orr, oi], axis=-1).reshape(t.shape)

    q = rope(q.reshape(S_, HQ_, D_))
    k = rope(k.reshape(S_, 1, D_))[:, 0]
    out = np.zeros((S_, HQ_ * D_))
    causal = np.tril(np.ones((S_, S_), dtype=bool))
    for m in range(HQ_):
        sc = q[:, m] @ k.T / math.sqrt(D_)
        sc = np.where(causal, sc, -np.inf)
        sc = sc - sc.max(axis=-1, keepdims=True)
        p = np.exp(sc)
        p /= p.sum(axis=-1, keepdims=True)
        out[:, m * D_:(m + 1) * D_] = p @ v
    y = out @ woT.astype(np.float64)  # [S, E]
    return np.ascontiguousarray(y.T.astype(np.float32))  # yT


# ---------------------------------------------------------------------------
# Entry point
# ---------------------------------------------------------------------------

_NC_CACHE = {}


def _get_nc():
    if "nc" not in _NC_CACHE:
        _NC_CACHE["nc"] = build_nc()
    return _NC_CACHE["nc"]


def kernel(x, wq_w, wk_w, wv_w, wo_w, freqs_cos, freqs_sin, start_pos=0,
           **_ignored):
    from concourse.bass_utils import run_bass_kernel_spmd

    nc = _get_nc()
    in_maps = shard_inputs(x, wq_w, wk_w, wv_w, wo_w, freqs_cos, freqs_sin)
    res = run_bass_kernel_spmd(nc, in_maps, list(range(N_CORES)))
    return gather_output(res.results)
